# revision 8
# baseline (speedup 1.0000x reference)
"""Trainium2 Bass kernel for nn_LossUResidu (7-pt Neumann Laplacian residual loss).

Math (derived from the reference):
  f = (input != 0), w = 1 - f = (input == 0)
  res = |lap3d(u)| * w / mx   per sample, mx = max(diag + fde*f)
  loss = mean over (B, nx*ny*nz)  [the b term and `interior` mask are
         irrelevant: b != 0 only where f == 1, where (1-f) zeroes res]

Sharding: 8 cores = 4 samples x 2 y-halves (pure data parallel). Host ships,
per core: y/z-halo-padded u (Neumann edges as clamp-duplication), x-shifted
copies of u (replication, so the partition-axis stencil needs no on-device
shift), and the w mask in bf16. Host combines tiny per-core partials.

Device program (SPMD, identical on all cores) — instruction-count-minimal
(this environment costs ~35us/instruction, so few huge ops win):
  xs = uxm + uxp                      (DVE, one 1M-elem op)
  zs = u<<z + u>>z                    (shifted APs into the padded tile)
  xz = xs + zs
  ys = u<<y + u>>y
  t  = -6*u + ys                      (scalar_tensor_tensor)
  L  = xz + t
  absl = |L|                          (ACT)
  ssum = accum((absl + 0) * w)        (scalar_tensor_tensor accum_out)
  rowtot = reduce_sum_z(w)            (for the honest max over diag + fde*f)
  zb = w[z=0] + w[z=127]
"""

import os
import sys

for _p in ("/root/.axon_site", "/root/.axon_site/_ro/trn_rl_repo",
           "/root/.axon_site/_ro/pypackages", "/opt/trn_rl_repo", "/opt/pypackages"):
    if os.path.isdir(_p) and _p not in sys.path:
        sys.path.append(_p)

import numpy as np
import ml_dtypes
from contextlib import ExitStack

import concourse.bacc as bacc
import concourse.tile as tile
from concourse import mybir
from concourse.bass_utils import run_bass_kernel_spmd

NXYZ = 128
B = 4
NCORES = 8
HALF = 64            # y-rows per core
FDE = float(np.float32((2.0 / (NXYZ - 1)) ** 2 / 1e-6))
P = 128

LAST_RESULTS = None
KREPS = int(os.environ.get("KREPS", "1"))

AOP = mybir.AluOpType


def _build_program():
    nc = bacc.Bacc("TRN2", target_bir_lowering=False, debug=False,
                   num_devices=NCORES)

    f32 = mybir.dt.float32
    u_d = nc.dram_tensor("u", [P, HALF + 2, NXYZ + 2], f32, kind="ExternalInput")
    uxm_d = nc.dram_tensor("uxm", [P, HALF, NXYZ], f32, kind="ExternalInput")
    uxp_d = nc.dram_tensor("uxp", [P, HALF, NXYZ], f32, kind="ExternalInput")
    w_d = nc.dram_tensor("w", [P, HALF, NXYZ], mybir.dt.bfloat16,
                         kind="ExternalInput")
    # packed outputs: col 0 = ssum, cols 1:65 = rowtot, cols 65:129 = zb
    out_d = nc.dram_tensor("out", [P, 1 + 2 * HALF], f32, kind="ExternalOutput")

    with tile.TileContext(nc) as tc:
        with ExitStack() as ctx:
            pool = ctx.enter_context(tc.tile_pool(name="main", bufs=1))
            small = ctx.enter_context(tc.tile_pool(name="small", bufs=1))

            u_t = pool.tile([P, HALF + 2, NXYZ + 2], f32)
            uxm_t = pool.tile([P, HALF, NXYZ], f32)
            uxp_t = pool.tile([P, HALF, NXYZ], f32)
            w_t = pool.tile([P, HALF, NXYZ], mybir.dt.bfloat16)
            t1 = pool.tile([P, HALF, NXYZ], f32)
            t2 = pool.tile([P, HALF, NXYZ], f32)

            out_t = small.tile([P, 1 + 2 * HALF], f32)
            ssum_t = out_t[:, 0:1]
            rt_t = out_t[:, 1:1 + HALF]
            zb_t = out_t[:, 1 + HALF:1 + 2 * HALF]

            for _rep in range(KREPS):
                nc.sync.dma_start(u_t[:], u_d[:])
                nc.sync.dma_start(uxm_t[:], uxm_d[:])
                nc.sync.dma_start(uxp_t[:], uxp_d[:])
                nc.sync.dma_start(w_t[:], w_d[:])

                # xs = uxm + uxp  -> t1
                nc.vector.scalar_tensor_tensor(
                    out=t1[:], in0=uxm_t[:], scalar=0.0, in1=uxp_t[:],
                    op0=AOP.add, op1=AOP.add)
                # zs = u<<z + u>>z -> uxm tile (dead)
                nc.vector.scalar_tensor_tensor(
                    out=uxm_t[:], in0=u_t[:, 1:HALF + 1, 0:NXYZ],
                    scalar=0.0, in1=u_t[:, 1:HALF + 1, 2:NXYZ + 2],
                    op0=AOP.add, op1=AOP.add)
                # xz = xs + zs -> uxp tile (dead)
                nc.vector.scalar_tensor_tensor(
                    out=uxp_t[:], in0=t1[:], scalar=0.0, in1=uxm_t[:],
                    op0=AOP.add, op1=AOP.add)
                # ys = u<<y + u>>y -> t1 (xs dead)
                nc.vector.scalar_tensor_tensor(
                    out=t1[:], in0=u_t[:, 0:HALF, 1:NXYZ + 1],
                    scalar=0.0, in1=u_t[:, 2:HALF + 2, 1:NXYZ + 1],
                    op0=AOP.add, op1=AOP.add)
                # t = -6*u + ys -> t2
                nc.vector.scalar_tensor_tensor(
                    out=t2[:], in0=u_t[:, 1:HALF + 1, 1:NXYZ + 1],
                    scalar=-6.0, in1=t1[:], op0=AOP.mult, op1=AOP.add)
                # L = xz + t -> t1
                nc.vector.scalar_tensor_tensor(
                    out=t1[:], in0=uxp_t[:], scalar=0.0, in1=t2[:],
                    op0=AOP.add, op1=AOP.add)
                # absl = |L| -> t2
                nc.scalar.activation(out=t2[:], in_=t1[:],
                                     func=mybir.ActivationFunctionType.Abs)
                # ssum = accum((absl + 0) * w) -> trash t1
                nc.vector.scalar_tensor_tensor(
                    out=t1[:], in0=t2[:], scalar=0.0, in1=w_t[:],
                    op0=AOP.add, op1=AOP.mult, accum_out=ssum_t)
                # max-path stats
                nc.vector.tensor_reduce(
                    out=rt_t, in_=w_t[:], axis=mybir.AxisListType.X,
                    op=AOP.add)
                nc.vector.scalar_tensor_tensor(
                    out=zb_t,
                    in0=w_t[:, :, 0:1].rearrange("p a b -> p (a b)"),
                    scalar=0.0,
                    in1=w_t[:, :, NXYZ - 1:NXYZ].rearrange("p a b -> p (a b)"),
                    op0=AOP.add, op1=AOP.add)

                nc.sync.dma_start(out_d[:], out_t[:])

    nc.finalize()
    return nc


_NC_CACHE = None


def _get_nc():
    global _NC_CACHE
    if _NC_CACHE is None:
        _NC_CACHE = _build_program()
    return _NC_CACHE


def _host_inputs(output, input):
    xm = np.maximum(np.arange(P) - 1, 0)
    xp = np.minimum(np.arange(P) + 1, P - 1)
    in_maps = []
    for c in range(NCORES):
        b, h = divmod(c, 2)
        u3 = np.asarray(output[b, 0], dtype=np.float32)          # (x,y,z)
        ysl = slice(HALF * h, HALF * (h + 1))
        uz = np.empty((P, P, NXYZ + 2), dtype=np.float32)
        uz[:, :, 1:NXYZ + 1] = u3
        uz[:, :, 0] = u3[:, :, 0]
        uz[:, :, NXYZ + 1] = u3[:, :, NXYZ - 1]
        yidx = np.clip(np.arange(HALF * h - 1, HALF * (h + 1) + 1), 0, P - 1)
        up = np.ascontiguousarray(uz[:, yidx, :])                # (128,66,130)
        uxm = np.ascontiguousarray(u3[xm][:, ysl, :])
        uxp = np.ascontiguousarray(u3[xp][:, ysl, :])
        w = np.ascontiguousarray(
            (np.asarray(input[b, 0, :, ysl, :]) == 0)
        ).astype(ml_dtypes.bfloat16)
        in_maps.append({"u": up, "uxm": uxm, "uxp": uxp, "w": w})
    return in_maps


def _host_combine(results):
    dvx = np.where((np.arange(P) == 0) | (np.arange(P) == P - 1), 1.0, 2.0)
    total = 0.0
    for b in range(B):
        s_b = 0.0
        m_b = 0.0
        for h in range(2):
            r = results[2 * b + h]["out"].astype(np.float64)
            s_b += float(r[:, 0].sum())
            rt = r[:, 1:1 + HALF]                  # (128, 64) per-row w counts
            zb = r[:, 1 + HALF:1 + 2 * HALF]       # (128, 64) w[z=0]+w[z=127]
            gy = np.arange(HALF) + HALF * h
            dvy = np.where((gy == 0) | (gy == NXYZ - 1), 1.0, 2.0)[None, :]
            dvxy = dvx[:, None] + dvy
            any_zi = (rt - zb) < 125.5             # needle in z-interior
            any_zb = zb < 1.5                      # needle in z-boundary cols
            f32 = np.float32
            cand_zi = np.where(any_zi, f32(FDE) + (dvxy + 2.0).astype(f32),
                               dvxy + 2.0)
            cand_zb = np.where(any_zb, f32(FDE) + (dvxy + 1.0).astype(f32),
                               dvxy + 1.0)
            m_b = max(m_b, float(cand_zi.max()), float(cand_zb.max()))
        total += s_b / m_b
    return np.float32(total / (B * NXYZ ** 3))


def kernel(output, gt, input):
    global LAST_RESULTS
    nc = _get_nc()
    in_maps = _host_inputs(output, input)
    res = run_bass_kernel_spmd(nc, in_maps, core_ids=list(range(NCORES)))
    LAST_RESULTS = res
    return _host_combine(res.results)
